# revision 1
# baseline (speedup 1.0000x reference)
"""CNF vector-field + exact Jacobian-trace kernel for Trainium2 (8 NeuronCores).

Math: for each sample x (D=32), with inp = [x, t] (33,):
  h1 = tanh(inp @ W1 + b1); h2 = tanh(h1 @ W2 + b2); dx = h2 @ W3 + b3
  div = trace(J),  J = W1r D1 W2 D2 W3  (D_i = diag(1 - h_i^2), W1r = W1[:32])
      = d1^T C d2,  C = W2 * (W3 @ W1r)^T   (elementwise *)
  out = [dx, div]  (B, 33)

Implementation notes:
  - data-parallel over batch (2048 -> 8 x 256), weights replicated
  - feature-major on-device layout: weights are natural pre-transposed lhsT
  - P = -C;  gt = P^T h1sq - (P^T 1);  E = (h2sq - 1) * gt = gt * d2 * (-1)
    div = (-1)^T E  -- the "1 - x^2" affines fold into matmuls / fused DVE ops
  - matmuls run as float32r (TF32-like, 4x faster than fp32 at N>=256)
  - consolidated DMAs via 3-D access patterns; W2 (the big one) issued last
  - engine streams are in-order: emission order is tuned so PE/ACT/DVE/Pool
    overlap (P-chain early, vp after z2, copies on ACT, h2sq on GpSimd)
"""
import sys

for _p in ("/opt/trn_rl_repo", "/root/.axon_site/_ro/trn_rl_repo"):
    if _p not in sys.path:
        sys.path.append(_p)

import numpy as np

B, D, H = 2048, 32, 512
NCORES = 8
BC = B // NCORES          # 256 rows per core
NK = H // 128             # 4 chunks of the hidden dim

_CACHE = {}


def _build(reps=None):
    import contextlib
    import concourse.bass as bass
    import concourse.tile as tile
    from concourse import bacc, mybir
    from concourse.masks import make_identity

    f32 = mybir.dt.float32
    f32r = mybir.dt.float32r
    AF = mybir.ActivationFunctionType
    ALU = mybir.AluOpType

    nc = bacc.Bacc("TRN2", target_bir_lowering=False, debug=False,
                   num_devices=NCORES)

    x_ext = nc.dram_tensor("x", [BC, D + 1], f32, kind="ExternalInput").ap()
    # w1 = [W1; b1] stacked then column-interleaved on host -> (16, 34, 32):
    # w1i[a, r, b] = w1s[r, a*32 + b]. The interleave makes the DMA split
    # into 34*16 non-contiguous descriptors so all 16 HWDGE queues are busy
    # (DMAs that leave queues empty get ~4us-late completion semaphores).
    w1_ext = nc.dram_tensor("w1", [16, D + 2, 32], f32r, kind="ExternalInput").ap()
    w2_ext = nc.dram_tensor("w2", [H, H], f32r, kind="ExternalInput").ap()
    w3_ext = nc.dram_tensor("w3", [H, D], f32r, kind="ExternalInput").ap()
    # colpack cols: 0=+1, 1=-1, 2=unused, 3:7=b2 column-major,
    # 7:11 = bias1 = t*W1[32,:]+b1 column-major (host-derived weight constant)
    colp_ext = nc.dram_tensor("colp", [128, 11], f32r, kind="ExternalInput").ap()
    # rowpack: [0:256]=ones, [256:288]=b3; host-padded to (16, 32) rows with
    # data in cols 0:18 so the DMA emits 16 strided descriptors (see w1 note)
    rowp_ext = nc.dram_tensor("rowp", [16, 32], f32r, kind="ExternalInput").ap()
    out_ext = nc.dram_tensor("out", [BC, D + 1], f32, kind="ExternalOutput").ap()

    with tile.TileContext(nc) as tc:
        with tc.tile_pool(name="const", bufs=1) as cpool, \
             tc.tile_pool(name="work", bufs=1) as wpool, \
             tc.tile_pool(name="ps", bufs=1, space="PSUM") as pps, \
             (tc.For_i(0, reps, 1) if reps else contextlib.nullcontext()):

            def big_ps(nm):
                return pps.tile([128, H], f32, name=nm, tag="big", bufs=6)

            def small_ps(nm, shape):
                return pps.tile(shape, f32, name=nm, tag="small", bufs=2)

            # -------- ACT spline-table preload (overlaps the DMA phase) -----
            dm0 = wpool.tile([1, 1], f32, name="dm0")
            dm1 = wpool.tile([1, 1], f32, name="dm1")
            nc.gpsimd.memset(dm0[:, :], 0.0)
            nc.scalar.activation(dm1[:, :], dm0[:, :], AF.Tanh)

            # ------------- input DMAs (few, large; W2 last) -------------
            w1e = cpool.tile([D + 2, H], f32r, name="w1e")   # 0:33 = W1, 33 = b1
            nc.sync.dma_start(
                out=w1e[:, :].rearrange("r (a b) -> r a b", a=16),
                in_=w1_ext.rearrange("a r b -> r a b"))

            colp = cpool.tile([128, 11], f32r, name="colp")
            nc.sync.dma_start(out=colp[:, :], in_=colp_ext[:, :])
            ones_col = colp[:, 0:1]
            neg_col = colp[:, 1:2]

            w3all = cpool.tile([128, NK * D], f32r, name="w3all")
            nc.sync.dma_start(
                out=w3all[:, :].rearrange("p (k j) -> p k j", k=NK),
                in_=w3_ext.rearrange("(k p) j -> p k j", k=NK))
            w3k = [w3all[:, k * D:(k + 1) * D] for k in range(NK)]

            xall = wpool.tile([128, 2 * (D + 1)], f32, name="xall")
            nc.scalar.dma_start(
                out=xall[:, :].rearrange("p (i c) -> p i c", i=2),
                in_=x_ext.rearrange("(i p) c -> p i c", i=2))

            w2all = cpool.tile([128, NK * H], f32r, name="w2all")
            nc.sync.dma_start(
                out=w2all[:, :].rearrange("p (k j) -> p k j", k=NK),
                in_=w2_ext.rearrange("(k p) j -> p k j", k=NK))
            w2k = [w2all[:, k * H:(k + 1) * H] for k in range(NK)]

            rowp = cpool.tile([1, BC + D], f32r, name="rowp")
            nc.sync.dma_start(
                out=rowp[:, :].rearrange("p (a b) -> p a b", a=16),
                in_=rowp_ext[:, 0:18].rearrange("(o a) b -> o a b", o=1))
            ones_row = rowp[:, 0:BC]
            b3row = rowp[:, BC:BC + D]

            ident = cpool.tile([128, 128], f32, name="ident")
            make_identity(nc, ident[:, :])

            # ------- W3^T (negated): PE transposes + DVE negate-copies -------
            negw3t = wpool.tile([D, H], f32r, name="negw3t")
            for k in range(NK):
                wp = small_ps("w3tp", [D, 128])
                nc.tensor.transpose(wp[:, :], w3k[k].bitcast(f32), ident[:, :])
                nc.vector.tensor_scalar(out=negw3t[:, k * 128:(k + 1) * 128],
                                        in0=wp[:, :], scalar1=-1.0, scalar2=None,
                                        op0=ALU.mult)

            # ---------------- x transpose: A0 = xs^T (32, 256) ----------------
            a0 = wpool.tile([D, BC], f32r, name="a0")
            for i in range(2):
                xp = small_ps("xT", [D + 1, 128])
                nc.tensor.transpose(xp[:, :], xall[:, i * (D + 1):(i + 1) * (D + 1)],
                                    ident[:, :])
                nc.vector.tensor_copy(a0[:, i * 128:(i + 1) * 128], xp[0:D, :])

            # ---------------- layer 1 matmuls, then all tanh ----------------
            z1s = []
            for m in range(NK):
                z1 = big_ps("z1")
                nc.tensor.matmul(z1[:, 0:BC], w1e[0:D, m * 128:(m + 1) * 128],
                                 a0[:, :], start=True, stop=True)
                z1s.append(z1)
            h1t = []
            for m in range(NK):
                h = wpool.tile([128, BC], f32r, name=f"h1t_{m}")
                nc.scalar.activation(h[:, :], z1s[m][:, 0:BC], AF.Tanh,
                                     bias=colp[:, 7 + m:8 + m].bitcast(f32))
                h1t.append(h)

            # ---------------- P = -(W2 * M^T), M = W3 @ W1r ----------------
            pmat = []
            for m in range(NK):
                mp = big_ps("negMt")
                nc.tensor.matmul(mp[:, :], w1e[0:D, m * 128:(m + 1) * 128],
                                 negw3t[:, :], start=True, stop=True)
                p = cpool.tile([128, H], f32r, name=f"p_{m}")
                nc.vector.tensor_tensor(out=p[:, :], in0=w2k[m].bitcast(f32),
                                        in1=mp[:, :], op=ALU.mult)
                pmat.append(p)

            # ---------------- vP row (early: gates the div tail) ------------
            vp_ps = small_ps("vp_ps", [1, H])
            for k in range(NK):
                nc.tensor.matmul(vp_ps[:, :], ones_col, pmat[k][:, :],
                                 start=(k == 0), stop=(k == NK - 1))
            vneg = wpool.tile([1, H], f32r, name="vneg")
            nc.scalar.activation(vneg[:, :], vp_ps[:, :], AF.Copy, scale=-1.0)

            # ---------------- h1sq on DVE (f32r, feeds gt matmuls) ----------
            h1sq = []
            for m in range(NK):
                sq = wpool.tile([128, BC], f32r, name=f"h1sq_{m}")
                nc.vector.tensor_tensor(out=sq[:, :], in0=h1t[m][:, :].bitcast(f32),
                                        in1=h1t[m][:, :].bitcast(f32), op=ALU.mult)
                h1sq.append(sq)

            # ---------------- layer 2 ----------------
            # k-outer so each z2[m] consumes h1t[k] as soon as tanh1[k] lands
            z2s = [big_ps("z2") for _ in range(NK)]
            for k in range(NK):
                for m in range(NK):
                    nc.tensor.matmul(z2s[m][:, 0:BC],
                                     w2k[k][:, m * 128:(m + 1) * 128],
                                     h1t[k][:, :],
                                     start=(k == 0), stop=(k == NK - 1))
            h2t = []
            for m in range(NK):
                h = wpool.tile([128, BC], f32r, name=f"h2t_{m}")
                nc.scalar.activation(h[:, :], z2s[m][:, 0:BC], AF.Tanh,
                                     bias=colp[:, 3 + m:4 + m].bitcast(f32))
                h2t.append(h)

            # ---------------- h2sq on GpSimd (SBUF only) ----------------
            h2sq = []
            for m in range(NK):
                sq = wpool.tile([128, BC], f32, name=f"h2sq_{m}")
                nc.gpsimd.tensor_tensor(out=sq[:, :], in0=h2t[m][:, :].bitcast(f32),
                                        in1=h2t[m][:, :].bitcast(f32), op=ALU.mult)
                h2sq.append(sq)

            # ------- gt = P^T h1sq - vP ; E = (h2sq - 1) * gt = -gt*d2 -------
            # k-outer gt accumulation, same early-consume pipelining
            gts = [big_ps("gt") for _ in range(NK)]
            for k in range(NK):
                for m in range(NK):
                    nc.tensor.matmul(gts[m][:, 0:BC],
                                     pmat[k][:, m * 128:(m + 1) * 128],
                                     h1sq[k][:, :],
                                     start=(k == 0), stop=False)
            ee = []
            for m in range(NK):
                nc.tensor.matmul(gts[m][:, 0:BC], vneg[:, m * 128:(m + 1) * 128],
                                 ones_row, start=False, stop=True)
                e = wpool.tile([128, BC], f32r, name=f"e_{m}")
                nc.vector.scalar_tensor_tensor(out=e[:, :], in0=h2sq[m][:, :],
                                               scalar=1.0, in1=gts[m][:, 0:BC],
                                               op0=ALU.subtract, op1=ALU.mult)
                ee.append(e)

            # -------- dx = W3^T h2 + b3 ; div = (-1)^T E --------
            dx_ps = small_ps("dx_ps", [D, BC])
            for k in range(NK):
                nc.tensor.matmul(dx_ps[:, :], w3k[k], h2t[k][:, :],
                                 start=(k == 0), stop=False)
            nc.tensor.matmul(dx_ps[:, :], b3row, ones_row,
                             start=False, stop=True)
            outt = wpool.tile([D + 1, BC], f32, name="outt")
            nc.scalar.activation(outt[0:D, :], dx_ps[:, :], AF.Copy)
            div_ps = small_ps("div_ps", [1, BC])
            for k in range(NK):
                nc.tensor.matmul(div_ps[:, :], neg_col, ee[k][:, :],
                                 start=(k == 0), stop=(k == NK - 1))
            nc.scalar.activation(outt[D:D + 1, :], div_ps[:, :], AF.Copy)

            # ------- transpose back to (256, 33) and store -------
            outs = wpool.tile([128, 2 * (D + 1)], f32, name="outs")
            for i in range(2):
                op = small_ps("outP", [128, D + 1])
                nc.tensor.transpose(op[:, :], outt[:, i * 128:(i + 1) * 128],
                                    ident[0:D + 1, 0:D + 1])
                nc.scalar.activation(outs[:, i * (D + 1):(i + 1) * (D + 1)],
                                     op[:, :], AF.Copy)
            nc.scalar.dma_start(
                out=out_ext.rearrange("(i p) c -> p i c", i=2),
                in_=outs[:, :].rearrange("p (i c) -> p i c", i=2))

    nc.compile()
    return nc


def _get_nc():
    if "nc" not in _CACHE:
        _CACHE["nc"] = _build()
    return _CACHE["nc"]


def _prep_inputs(t, x, W1, b1, W2, b2, W3, b3):
    t = np.asarray(t, dtype=np.float32)
    x = np.ascontiguousarray(np.asarray(x, dtype=np.float32))
    W1 = np.asarray(W1, dtype=np.float32)
    b1 = np.asarray(b1, dtype=np.float32)
    w1s = np.concatenate([W1, b1.reshape(1, H)], axis=0)
    w1s = np.ascontiguousarray(
        w1s.reshape(D + 2, 16, 32).transpose(1, 0, 2))  # (16, 34, 32)
    W2 = np.ascontiguousarray(np.asarray(W2, dtype=np.float32))
    W3 = np.ascontiguousarray(np.asarray(W3, dtype=np.float32))
    colp = np.zeros((128, 11), dtype=np.float32)
    colp[:, 0] = 1.0
    colp[:, 1] = -1.0
    colp[:, 3:7] = np.asarray(b2, dtype=np.float32).reshape(NK, 128).T
    bias1 = (np.float32(t.ravel()[0]) * W1[D, :] + b1).astype(np.float32)
    colp[:, 7:11] = bias1.reshape(NK, 128).T
    rowv = np.ones(BC + D, dtype=np.float32)
    rowv[BC:] = np.asarray(b3, dtype=np.float32)
    rowp = np.zeros((16, 32), dtype=np.float32)
    rowp[:, 0:18] = rowv.reshape(16, 18)
    return x, w1s, W2, W3, colp, rowp


def kernel(t, x, W1, b1, W2, b2, W3, b3):
    from concourse.bass_utils import run_bass_kernel_spmd

    nc = _get_nc()
    x, w1s, W2, W3, colp, rowp = _prep_inputs(t, x, W1, b1, W2, b2, W3, b3)
    in_maps = []
    for i in range(NCORES):
        in_maps.append({
            "x": np.ascontiguousarray(x[i * BC:(i + 1) * BC]),
            "w1": w1s, "w2": W2, "w3": W3,
            "colp": colp, "rowp": rowp,
        })
    res = run_bass_kernel_spmd(nc, in_maps, core_ids=list(range(NCORES)))
    return np.concatenate([res.results[i]["out"] for i in range(NCORES)], axis=0)



# revision 3
# speedup vs baseline: 1.1381x; 1.1381x over previous
"""CNF vector-field + exact Jacobian-trace kernel for Trainium2 (8 NeuronCores).

Math: for each sample x (D=32), with inp = [x, t] (33,):
  h1 = tanh(inp @ W1 + b1); h2 = tanh(h1 @ W2 + b2); dx = h2 @ W3 + b3
  div = trace(J) = d1^T C d2,  C = W2 * (W3 @ W1r)^T,  d_i = 1 - h_i^2
  out = [dx, div]  (B, 33)

Implementation notes (hardware-measured constraints):
  - all matmul operands bf16 (tol 2e-2, measured ~4e-3): single-pass PE
    matmuls (fp32r is 2-pass) and half the DMA bytes
  - PSUM accumulation groups must NOT share a bank: a group's start=True
    clears the whole bank's has_written bits, so an interleaved second
    group makes the first overwrite instead of accumulate. One group per
    2KB bank; an 8-slot ring recycles banks (warm/z1 -> mp -> z2 -> gt ->
    dx/div) in dependency order.
  - DMA descriptors must be <=1KB and >=128 per transfer: >2KB descriptors
    serialize onto ~2 of the 16 DMA engines (41 GB/s vs 151 GB/s), and a
    clogged engine also delays every queued completion-semaphore write.
  - host pre-computes: x^T with a ones row (bias1 via K=33 matmul row),
    -W3^T, W3 row-chunk pack, vneg = colsum(W2 * M^T); output goes out
    untransposed and the host transposes back.
  - PE warm-up matmuls bridge the DMA wait: the HAM clock gate runs the PE
    at 1.2 GHz until ~3.4us of sustained activity, 2.4 GHz after.
"""
import sys

for _p in ("/opt/trn_rl_repo", "/root/.axon_site/_ro/trn_rl_repo"):
    if _p not in sys.path:
        sys.path.append(_p)

import numpy as np
import ml_dtypes

B, D, H = 2048, 32, 512
NCORES = 8
BC = B // NCORES          # 256 rows per core
NK = H // 128             # 4 chunks of the hidden dim
BF = ml_dtypes.bfloat16

# pack_a column offsets (bf16, 34 partitions); DMA'd with a 4-way column
# interleave so each 2560B row becomes 4 x 640B descriptors
XT = 0                    # [0:33, 0:256]   xs^T rows 0:32, ones row 32
W1B = BC                  # [0:33, 256:768] W1r rows 0:32, bias1 row 32
NW = BC + H               # [0:32, 768:1280] -W3^T
PACKC = BC + 2 * H        # 1280
PAI = 4                   # pack interleave factor
# brow offsets (bf16, partition 0, 1056 = 16*66 elems)
B2O, VNO, B3O = 0, H, 2 * H

_CACHE = {}


def _build():
    import concourse.bass as bass  # noqa: F401
    import concourse.tile as tile
    from concourse import bacc, mybir

    f32 = mybir.dt.float32
    bf16 = mybir.dt.bfloat16
    AF = mybir.ActivationFunctionType
    ALU = mybir.AluOpType

    nc = bacc.Bacc("TRN2", target_bir_lowering=False, debug=False,
                   num_devices=NCORES)

    pack_ext = nc.dram_tensor("packa", [PAI, D + 2, PACKC // PAI], bf16,
                              kind="ExternalInput").ap()
    w2_ext = [nc.dram_tensor(f"w2k{k}", [128, H], bf16,
                             kind="ExternalInput").ap() for k in range(NK)]
    w3p_ext = nc.dram_tensor("w3p", [128, NK * D], bf16,
                             kind="ExternalInput").ap()
    brow_ext = nc.dram_tensor("brow", [16, 66], bf16,
                              kind="ExternalInput").ap()
    odx_ext = nc.dram_tensor("out_dx", [D, BC], f32,
                             kind="ExternalOutput").ap()
    odiv_ext = nc.dram_tensor("out_div", [1, BC], f32,
                              kind="ExternalOutput").ap()

    with tile.TileContext(nc) as tc:
        with tc.tile_pool(name="const", bufs=1) as cpool, \
             tc.tile_pool(name="work", bufs=1) as wpool, \
             tc.tile_pool(name="ps", bufs=1, space="PSUM") as pps:

            def ps_tile(nm, shape=(128, H)):
                return pps.tile(list(shape), f32, name=nm, tag="ring", bufs=8)

            # ---- on-device constants (no DMA) + ACT table preload ----
            wsrc = wpool.tile([128, H], bf16, name="wsrc")
            nc.gpsimd.memset(wsrc[:, :], 0.0)
            ones_row = wpool.tile([1, BC], bf16, name="ones_row")
            nc.gpsimd.memset(ones_row[:, :], 1.0)
            neg_col = wpool.tile([128, 1], bf16, name="neg_col")
            nc.gpsimd.memset(neg_col[:, :], -1.0)
            dm0 = wpool.tile([1, 1], f32, name="dm0")
            nc.gpsimd.memset(dm0[:, :], 0.0)
            dm1 = wpool.tile([1, 1], f32, name="dm1")
            nc.scalar.activation(dm1[:, :], dm0[:, :], AF.Tanh)

            # ---- input DMAs, interleaved across the two HWDGE queues ----
            packa = cpool.tile([D + 2, PACKC], bf16, name="packa")
            nc.scalar.dma_start(
                out=packa[:, :].rearrange("r (a b) -> r a b", a=PAI),
                in_=pack_ext.rearrange("a r b -> r a b"))
            w2k = [cpool.tile([128, H], bf16, name=f"w2k{k}")
                   for k in range(NK)]
            nc.sync.dma_start(out=w2k[0][:, :], in_=w2_ext[0][:, :])
            nc.scalar.dma_start(out=w2k[1][:, :], in_=w2_ext[1][:, :])
            nc.sync.dma_start(out=w2k[2][:, :], in_=w2_ext[2][:, :])
            nc.scalar.dma_start(out=w2k[3][:, :], in_=w2_ext[3][:, :])
            brow = cpool.tile([1, 16 * 66], bf16, name="brow")
            nc.sync.dma_start(
                out=brow[:, :].rearrange("p (a b) -> p a b", a=16),
                in_=brow_ext.rearrange("(o a) b -> o a b", o=1))
            w3p = cpool.tile([128, NK * D], bf16, name="w3p")
            nc.sync.dma_start(out=w3p[:, :], in_=w3p_ext[:, :])

            # ---- PE warm-up against the HAM clock gate (ring slots 0-5) --
            for i in range(5):
                wp = ps_tile(f"warm{i}")
                nc.tensor.matmul(wp[:, :], wsrc[:, 0:128], wsrc[:, :],
                                 start=True, stop=True)
            wp = ps_tile("warm5", shape=(128, BC))
            nc.tensor.matmul(wp[:, :], wsrc[:, 0:128], wsrc[:, 0:BC],
                             start=True, stop=True)

            # ---- z1 (K=33: bias1 folded in via the ones row of x^T) ----
            # ring slots 6,7; two single-MM groups per bank is safe (each
            # is start+stop in one instruction)
            a0 = packa[0:D + 1, XT:XT + BC]
            z1t = [ps_tile(f"z1{i}") for i in range(2)]
            for m in range(NK):
                nc.tensor.matmul(
                    z1t[m // 2][:, (m % 2) * BC:(m % 2 + 1) * BC],
                    packa[0:D + 1, W1B + m * 128:W1B + (m + 1) * 128],
                    a0, start=True, stop=True)
            h1t = [wpool.tile([128, 2 * BC], bf16, name=f"h1t{i}")
                   for i in range(2)]
            for i in range(2):
                nc.scalar.activation(h1t[i][:, :], z1t[i][:, :], AF.Tanh)

            # ---- mp = W1r^T @ (-W3^T) per row-chunk (slots 0-3);
            #      P = W2 * mp on DVE, interleaved with h1sq ----
            pmat = [cpool.tile([128, H], bf16, name=f"p{k}")
                    for k in range(NK)]
            h1sq = [wpool.tile([128, 2 * BC], bf16, name=f"h1sq{i}")
                    for i in range(2)]
            mps = [ps_tile(f"mp{k}") for k in range(NK)]
            for k in range(NK):
                nc.tensor.matmul(mps[k][:, :],
                                 packa[0:D, W1B + k * 128:W1B + (k + 1) * 128],
                                 packa[0:D, NW:NW + H], start=True, stop=True)
            nc.vector.tensor_tensor(out=h1sq[0][:, :], in0=h1t[0][:, :],
                                    in1=h1t[0][:, :], op=ALU.mult)
            for k in range(NK):
                nc.vector.tensor_tensor(out=pmat[k][:, :], in0=w2k[k][:, :],
                                        in1=mps[k][:, :], op=ALU.mult)
                if k == 1:
                    nc.vector.tensor_tensor(out=h1sq[1][:, :],
                                            in0=h1t[1][:, :],
                                            in1=h1t[1][:, :], op=ALU.mult)

            # ---- z2 & gt accumulation rounds (k-outer) ----
            # one group per bank: z2 slots 4,5,6,7 / gt slots 0,1,2,3
            z2t = [ps_tile(f"z2{m}", shape=(128, BC)) for m in range(NK)]
            gtt = [ps_tile(f"gt{m}", shape=(128, BC)) for m in range(NK)]
            for k in range(NK):
                hk = h1t[k // 2][:, (k % 2) * BC:(k % 2 + 1) * BC]
                sk = h1sq[k // 2][:, (k % 2) * BC:(k % 2 + 1) * BC]
                for m in range(NK):
                    nc.tensor.matmul(z2t[m][:, :],
                                     w2k[k][:, m * 128:(m + 1) * 128], hk,
                                     start=(k == 0), stop=False)
                if k == NK - 1:
                    # close z2 groups first so tanh2 overlaps gt round 3
                    for m in range(NK):
                        nc.tensor.matmul(z2t[m][:, :],
                                         brow[:, B2O + m * 128:B2O + (m + 1) * 128],
                                         ones_row, start=False, stop=True)
                for m in range(NK):
                    nc.tensor.matmul(gtt[m][:, :],
                                     pmat[k][:, m * 128:(m + 1) * 128], sk,
                                     start=(k == 0), stop=False)
                if k == NK - 1:
                    for m in range(NK):
                        nc.tensor.matmul(gtt[m][:, :],
                                         brow[:, VNO + m * 128:VNO + (m + 1) * 128],
                                         ones_row, start=False, stop=True)

            # ---- tanh2 per chunk, h2sq (GpSimd+DVE), E ----
            h2t = [wpool.tile([128, 2 * BC], bf16, name=f"h2t{i}")
                   for i in range(2)]
            for m in range(NK):
                nc.scalar.activation(h2t[m // 2][:, (m % 2) * BC:(m % 2 + 1) * BC],
                                     z2t[m][:, :], AF.Tanh)
            h2sq = [wpool.tile([128, 2 * BC], bf16, name=f"h2sq{i}")
                    for i in range(2)]
            for m in range(2):
                nc.gpsimd.tensor_tensor(
                    out=h2sq[m // 2][:, (m % 2) * BC:(m % 2 + 1) * BC],
                    in0=h2t[m // 2][:, (m % 2) * BC:(m % 2 + 1) * BC],
                    in1=h2t[m // 2][:, (m % 2) * BC:(m % 2 + 1) * BC],
                    op=ALU.mult)
            ee = [wpool.tile([128, 2 * BC], bf16, name=f"ee{i}")
                  for i in range(2)]
            for m in range(2, NK):
                nc.vector.tensor_tensor(
                    out=h2sq[m // 2][:, (m % 2) * BC:(m % 2 + 1) * BC],
                    in0=h2t[m // 2][:, (m % 2) * BC:(m % 2 + 1) * BC],
                    in1=h2t[m // 2][:, (m % 2) * BC:(m % 2 + 1) * BC],
                    op=ALU.mult)
            for m in range(NK):
                nc.vector.scalar_tensor_tensor(
                    out=ee[m // 2][:, (m % 2) * BC:(m % 2 + 1) * BC],
                    in0=h2sq[m // 2][:, (m % 2) * BC:(m % 2 + 1) * BC],
                    scalar=1.0, in1=gtt[m][:, :],
                    op0=ALU.subtract, op1=ALU.mult)

            # ---- dx = W3^T h2 + b3 (slot 4); div = (-1)^T E (slot 5) ----
            dx_ps = ps_tile("dx", shape=(D, BC))
            for k in range(NK):
                nc.tensor.matmul(dx_ps[:, :], w3p[:, k * D:(k + 1) * D],
                                 h2t[k // 2][:, (k % 2) * BC:(k % 2 + 1) * BC],
                                 start=(k == 0), stop=False)
            nc.tensor.matmul(dx_ps[:, :], brow[:, B3O:B3O + D], ones_row,
                             start=False, stop=True)
            div_ps = ps_tile("div", shape=(1, BC))
            for k in range(NK):
                nc.tensor.matmul(div_ps[:, :], neg_col[:, :],
                                 ee[k // 2][:, (k % 2) * BC:(k % 2 + 1) * BC],
                                 start=(k == 0), stop=(k == NK - 1))

            # ---- stage on DVE (ACT is busy with tanh2), store on both
            #      queues in parallel ----
            odx = wpool.tile([D, BC], f32, name="odx")
            nc.vector.tensor_copy(odx[:, :], dx_ps[:, :])
            nc.sync.dma_start(out=odx_ext[:, :], in_=odx[:, :])
            odiv = wpool.tile([1, BC], f32, name="odiv")
            nc.vector.tensor_copy(odiv[:, :], div_ps[:, :])
            nc.scalar.dma_start(out=odiv_ext[:, :], in_=odiv[:, :])

    nc.compile()
    return nc


def _get_nc():
    if "nc" not in _CACHE:
        _CACHE["nc"] = _build()
    return _CACHE["nc"]


def _make_in_maps(t, x, W1, b1, W2, b2, W3, b3):
    t0 = np.float32(np.asarray(t, np.float32).ravel()[0])
    x = np.asarray(x, np.float32)
    W1 = np.asarray(W1, np.float32)
    b1 = np.asarray(b1, np.float32)
    W2 = np.asarray(W2, np.float32)
    b2 = np.asarray(b2, np.float32)
    W3 = np.asarray(W3, np.float32)
    b3 = np.asarray(b3, np.float32)

    bias1 = t0 * W1[D] + b1
    w1b = np.concatenate([W1[:D], bias1[None, :]], axis=0)      # (33, 512)
    negw3t = -W3.T                                               # (32, 512)

    common = np.zeros((D + 2, PACKC), dtype=BF)
    common[0:D + 1, W1B:W1B + H] = w1b.astype(BF)
    common[0:D, NW:NW + H] = negw3t.astype(BF)

    w2ks = [np.ascontiguousarray(W2[k * 128:(k + 1) * 128]).astype(BF)
            for k in range(NK)]
    w3p = np.ascontiguousarray(
        W3.reshape(NK, 128, D).transpose(1, 0, 2).reshape(128, NK * D)
    ).astype(BF)

    Mt = (W3.astype(np.float64) @ W1[:D].astype(np.float64)).T   # M^T (H, H)
    vneg = (W2.astype(np.float64) * Mt).sum(axis=0)              # colsum of C
    v = np.zeros(16 * 66, dtype=np.float32)
    v[B2O:B2O + H] = b2
    v[VNO:VNO + H] = vneg.astype(np.float32)
    v[B3O:B3O + D] = b3
    brow = np.ascontiguousarray(v.astype(BF).reshape(16, 66))

    in_maps = []
    for i in range(NCORES):
        packa = common.copy()
        xs = x[i * BC:(i + 1) * BC, :D]
        packa[0:D, XT:XT + BC] = xs.T.astype(BF)
        packa[D, XT:XT + BC] = BF(1.0)
        # 4-way column interleave: row r -> 4 descriptors of 640B
        packi = np.ascontiguousarray(
            packa.reshape(D + 2, PAI, PACKC // PAI).transpose(1, 0, 2))
        m = {"packa": packi, "w3p": w3p, "brow": brow}
        for k in range(NK):
            m[f"w2k{k}"] = w2ks[k]
        in_maps.append(m)
    return in_maps


def kernel(t, x, W1, b1, W2, b2, W3, b3):
    from concourse.bass_utils import run_bass_kernel_spmd

    nc = _get_nc()
    in_maps = _make_in_maps(t, x, W1, b1, W2, b2, W3, b3)
    res = run_bass_kernel_spmd(nc, in_maps, core_ids=list(range(NCORES)))
    parts = []
    for i in range(NCORES):
        dx = res.results[i]["out_dx"]        # (32, 256)
        dv = res.results[i]["out_div"]       # (1, 256)
        parts.append(np.concatenate([dx.T, dv.T], axis=1))
    return np.ascontiguousarray(np.concatenate(parts, axis=0))


# revision 4
# speedup vs baseline: 1.1837x; 1.0401x over previous
"""CNF vector-field + exact Jacobian-trace kernel for Trainium2 (8 NeuronCores).

Math: for each sample x (D=32), with inp = [x, t] (33,):
  h1 = tanh(inp @ W1 + b1); h2 = tanh(h1 @ W2 + b2); dx = h2 @ W3 + b3
  div = trace(J) = d1^T C d2,  C = W2 * (W3 @ W1r)^T,  d_i = 1 - h_i^2
  out = [dx, div]  (B, 33)

Implementation notes (hardware-measured constraints):
  - all matmul operands bf16 (tol 2e-2, measured ~4e-3): single-pass PE
    matmuls (fp32r is 2-pass) and half the DMA bytes
  - PSUM accumulation groups must NOT share a bank: a group's start=True
    clears the whole bank's has_written bits, so an interleaved second
    group makes the first overwrite instead of accumulate. One group per
    2KB bank; an 8-slot ring recycles banks (warm/z1 -> mp -> z2 -> gt ->
    dx/div) in dependency order.
  - DMA engines cost ~125ns per descriptor: plain 2-D row-per-descriptor
    transfers spread round-robin over all 16 engines, and 2KB descriptors
    reach ~250 GB/s/queue (1KB ~150). Rearranged/3-D patterns serialize
    onto ~2 engines — avoid. W2 row-chunks are DMA'd as column-paired
    (128, 1024) tiles so each descriptor is 2KB.
  - completion semaphores ride the same engines as data: keep every
    descriptor <=2KB and all transfers >=16 descriptors so no engine
    clogs and sems arrive with the data.
  - host pre-computes: x^T with a ones row (bias1 via K=33 matmul row),
    -W3^T, W3 row-chunk pack, vneg = colsum(W2 * M^T); output goes out
    untransposed and the host transposes back.
  - PE warm-up matmuls bridge the DMA wait: the HAM clock gate runs the PE
    at 1.2 GHz until ~3.4us of sustained activity, 2.4 GHz after.
"""
import sys

for _p in ("/opt/trn_rl_repo", "/root/.axon_site/_ro/trn_rl_repo"):
    if _p not in sys.path:
        sys.path.append(_p)

import numpy as np
import ml_dtypes

B, D, H = 2048, 32, 512
NCORES = 8
BC = B // NCORES          # 256 rows per core
NK = H // 128             # 4 chunks of the hidden dim
BF = ml_dtypes.bfloat16

# brow offsets (bf16, partition 0, 1056 = 16*66 elems)
B2O, VNO, B3O = 0, H, 2 * H

_CACHE = {}


def _build():
    import concourse.bass as bass  # noqa: F401
    import concourse.tile as tile
    from concourse import bacc, mybir

    f32 = mybir.dt.float32
    bf16 = mybir.dt.bfloat16
    AF = mybir.ActivationFunctionType
    ALU = mybir.AluOpType

    nc = bacc.Bacc("TRN2", target_bir_lowering=False, debug=False,
                   num_devices=NCORES)

    xt_ext = nc.dram_tensor("xt", [D + 1, BC], bf16,
                            kind="ExternalInput").ap()
    w1b_ext = nc.dram_tensor("w1b", [D + 1, H], bf16,
                             kind="ExternalInput").ap()
    w2p_ext = [nc.dram_tensor(f"w2p{i}", [128, 2 * H], bf16,
                              kind="ExternalInput").ap() for i in range(2)]
    nw_ext = nc.dram_tensor("negw3t", [D, H], bf16,
                            kind="ExternalInput").ap()
    w3p_ext = nc.dram_tensor("w3p", [128, NK * D], bf16,
                             kind="ExternalInput").ap()
    brow_ext = nc.dram_tensor("brow", [16, 66], bf16,
                              kind="ExternalInput").ap()
    odx_ext = nc.dram_tensor("out_dx", [D, BC], f32,
                             kind="ExternalOutput").ap()
    odiv_ext = nc.dram_tensor("out_div", [1, BC], f32,
                              kind="ExternalOutput").ap()

    with tile.TileContext(nc) as tc:
        with tc.tile_pool(name="const", bufs=1) as cpool, \
             tc.tile_pool(name="work", bufs=1) as wpool, \
             tc.tile_pool(name="ps", bufs=1, space="PSUM") as pps:

            def ps_tile(nm, shape=(128, H)):
                return pps.tile(list(shape), f32, name=nm, tag="ring", bufs=8)

            # ---- on-device constants (no DMA) + ACT table preload ----
            wsrc = wpool.tile([128, H], bf16, name="wsrc")
            nc.gpsimd.memset(wsrc[:, :], 0.0)
            ones_row = wpool.tile([1, BC], bf16, name="ones_row")
            nc.gpsimd.memset(ones_row[:, :], 1.0)
            neg_col = wpool.tile([128, 1], bf16, name="neg_col")
            nc.gpsimd.memset(neg_col[:, :], -1.0)
            dm0 = wpool.tile([1, 1], f32, name="dm0")
            nc.gpsimd.memset(dm0[:, :], 0.0)
            dm1 = wpool.tile([1, 1], f32, name="dm1")
            nc.scalar.activation(dm1[:, :], dm0[:, :], AF.Tanh)

            # ---- input DMAs: plain 2-D only, split across both queues ----
            # scalar queue: z1 inputs first, then W2 pair 1
            w1b = cpool.tile([D + 1, H], bf16, name="w1b")
            nc.scalar.dma_start(out=w1b[:, :], in_=w1b_ext[:, :])
            xt = cpool.tile([D + 1, BC], bf16, name="xt")
            nc.scalar.dma_start(out=xt[:, :], in_=xt_ext[:, :])
            # sync queue: W2 pair 0 first, then mp/dx inputs
            w2p = [cpool.tile([128, 2 * H], bf16, name=f"w2p{i}")
                   for i in range(2)]
            nc.sync.dma_start(out=w2p[0][:, :], in_=w2p_ext[0][:, :])
            nc.scalar.dma_start(out=w2p[1][:, :], in_=w2p_ext[1][:, :])
            negw3t = cpool.tile([D, H], bf16, name="negw3t")
            nc.sync.dma_start(out=negw3t[:, :], in_=nw_ext[:, :])
            w3p = cpool.tile([128, NK * D], bf16, name="w3p")
            nc.sync.dma_start(out=w3p[:, :], in_=w3p_ext[:, :])
            brow = cpool.tile([1, 16 * 66], bf16, name="brow")
            nc.sync.dma_start(
                out=brow[:, :].rearrange("p (a b) -> p a b", a=16),
                in_=brow_ext.rearrange("(o a) b -> o a b", o=1))
            w2k = [w2p[k // 2][:, (k % 2) * H:(k % 2 + 1) * H]
                   for k in range(NK)]

            # ---- PE warm-up against the HAM clock gate (ring slots 0-5) --
            for i in range(5):
                wp = ps_tile(f"warm{i}")
                nc.tensor.matmul(wp[:, :], wsrc[:, 0:128], wsrc[:, :],
                                 start=True, stop=True)
            wp = ps_tile("warm5", shape=(128, 64))
            nc.tensor.matmul(wp[:, :], wsrc[:, 0:128], wsrc[:, 0:64],
                             start=True, stop=True)

            # ---- z1 (K=33: bias1 folded in via the ones row of x^T) ----
            # ring slots 6,7; two single-MM groups per bank is safe (each
            # is start+stop in one instruction)
            z1t = [ps_tile(f"z1{i}") for i in range(2)]
            for m in range(NK):
                nc.tensor.matmul(
                    z1t[m // 2][:, (m % 2) * BC:(m % 2 + 1) * BC],
                    w1b[:, m * 128:(m + 1) * 128],
                    xt[:, :], start=True, stop=True)
            h1t = [wpool.tile([128, 2 * BC], bf16, name=f"h1t{i}")
                   for i in range(2)]
            for i in range(2):
                nc.scalar.activation(h1t[i][:, :], z1t[i][:, :], AF.Tanh)

            # ---- mp = W1r^T @ (-W3^T) per row-chunk (slots 0-3);
            #      P = W2 * mp on DVE, interleaved with h1sq ----
            pmat = [cpool.tile([128, H], bf16, name=f"p{k}")
                    for k in range(NK)]
            h1sq = [wpool.tile([128, 2 * BC], bf16, name=f"h1sq{i}")
                    for i in range(2)]
            mps = [ps_tile(f"mp{k}") for k in range(NK)]
            for k in range(NK):
                nc.tensor.matmul(mps[k][:, :],
                                 w1b[0:D, k * 128:(k + 1) * 128],
                                 negw3t[:, :], start=True, stop=True)
            nc.vector.tensor_tensor(out=h1sq[0][:, :], in0=h1t[0][:, :],
                                    in1=h1t[0][:, :], op=ALU.mult)
            nc.vector.tensor_tensor(out=pmat[0][:, :], in0=w2k[0][:, :],
                                    in1=mps[0][:, :], op=ALU.mult)
            nc.vector.tensor_tensor(out=h1sq[1][:, :], in0=h1t[1][:, :],
                                    in1=h1t[1][:, :], op=ALU.mult)
            for k in range(1, NK):
                nc.vector.tensor_tensor(out=pmat[k][:, :], in0=w2k[k][:, :],
                                        in1=mps[k][:, :], op=ALU.mult)

            # ---- z2 & gt accumulation rounds (k-outer) ----
            # one group per bank: z2 slots 4,5,6,7 / gt slots 0,1,2,3
            z2t = [ps_tile(f"z2{m}", shape=(128, BC)) for m in range(NK)]
            gtt = [ps_tile(f"gt{m}", shape=(128, BC)) for m in range(NK)]
            for k in range(NK):
                hk = h1t[k // 2][:, (k % 2) * BC:(k % 2 + 1) * BC]
                sk = h1sq[k // 2][:, (k % 2) * BC:(k % 2 + 1) * BC]
                for m in range(NK):
                    nc.tensor.matmul(z2t[m][:, :],
                                     w2k[k][:, m * 128:(m + 1) * 128], hk,
                                     start=(k == 0), stop=False)
                if k == NK - 1:
                    # close z2 groups first so tanh2 overlaps gt round 3
                    for m in range(NK):
                        nc.tensor.matmul(z2t[m][:, :],
                                         brow[:, B2O + m * 128:B2O + (m + 1) * 128],
                                         ones_row, start=False, stop=True)
                for m in range(NK):
                    nc.tensor.matmul(gtt[m][:, :],
                                     pmat[k][:, m * 128:(m + 1) * 128], sk,
                                     start=(k == 0), stop=False)
                if k == NK - 1:
                    for m in range(NK):
                        nc.tensor.matmul(gtt[m][:, :],
                                         brow[:, VNO + m * 128:VNO + (m + 1) * 128],
                                         ones_row, start=False, stop=True)

            # ---- tanh2 per chunk, h2sq (GpSimd+DVE), E ----
            h2t = [wpool.tile([128, 2 * BC], bf16, name=f"h2t{i}")
                   for i in range(2)]
            for m in range(NK):
                nc.scalar.activation(h2t[m // 2][:, (m % 2) * BC:(m % 2 + 1) * BC],
                                     z2t[m][:, :], AF.Tanh)
            h2sq = [wpool.tile([128, 2 * BC], bf16, name=f"h2sq{i}")
                    for i in range(2)]
            for m in range(2):
                nc.gpsimd.tensor_tensor(
                    out=h2sq[m // 2][:, (m % 2) * BC:(m % 2 + 1) * BC],
                    in0=h2t[m // 2][:, (m % 2) * BC:(m % 2 + 1) * BC],
                    in1=h2t[m // 2][:, (m % 2) * BC:(m % 2 + 1) * BC],
                    op=ALU.mult)
            for m in range(2, NK):
                nc.vector.tensor_tensor(
                    out=h2sq[m // 2][:, (m % 2) * BC:(m % 2 + 1) * BC],
                    in0=h2t[m // 2][:, (m % 2) * BC:(m % 2 + 1) * BC],
                    in1=h2t[m // 2][:, (m % 2) * BC:(m % 2 + 1) * BC],
                    op=ALU.mult)
            ee = [wpool.tile([128, 2 * BC], bf16, name=f"ee{i}")
                  for i in range(2)]
            for m in range(NK):
                nc.vector.scalar_tensor_tensor(
                    out=ee[m // 2][:, (m % 2) * BC:(m % 2 + 1) * BC],
                    in0=h2sq[m // 2][:, (m % 2) * BC:(m % 2 + 1) * BC],
                    scalar=1.0, in1=gtt[m][:, :],
                    op0=ALU.subtract, op1=ALU.mult)

            # ---- dx = W3^T h2 + b3 (slot 4); div = (-1)^T E (slot 5) ----
            dx_ps = ps_tile("dx", shape=(D, BC))
            for k in range(NK):
                nc.tensor.matmul(dx_ps[:, :], w3p[:, k * D:(k + 1) * D],
                                 h2t[k // 2][:, (k % 2) * BC:(k % 2 + 1) * BC],
                                 start=(k == 0), stop=False)
            nc.tensor.matmul(dx_ps[:, :], brow[:, B3O:B3O + D], ones_row,
                             start=False, stop=True)
            div_ps = ps_tile("div", shape=(1, BC))
            for k in range(NK):
                nc.tensor.matmul(div_ps[:, :], neg_col[:, :],
                                 ee[k // 2][:, (k % 2) * BC:(k % 2 + 1) * BC],
                                 start=(k == 0), stop=(k == NK - 1))

            # ---- stage on DVE (ACT is busy with tanh2), store on both
            #      queues in parallel ----
            odx = wpool.tile([D, BC], f32, name="odx")
            nc.vector.tensor_copy(odx[:, :], dx_ps[:, :])
            nc.sync.dma_start(out=odx_ext[:, :], in_=odx[:, :])
            odiv = wpool.tile([1, BC], f32, name="odiv")
            nc.vector.tensor_copy(odiv[:, :], div_ps[:, :])
            nc.scalar.dma_start(out=odiv_ext[:, :], in_=odiv[:, :])

    nc.compile()
    return nc


def _get_nc():
    if "nc" not in _CACHE:
        _CACHE["nc"] = _build()
    return _CACHE["nc"]


def _make_in_maps(t, x, W1, b1, W2, b2, W3, b3):
    t0 = np.float32(np.asarray(t, np.float32).ravel()[0])
    x = np.asarray(x, np.float32)
    W1 = np.asarray(W1, np.float32)
    b1 = np.asarray(b1, np.float32)
    W2 = np.asarray(W2, np.float32)
    b2 = np.asarray(b2, np.float32)
    W3 = np.asarray(W3, np.float32)
    b3 = np.asarray(b3, np.float32)

    bias1 = t0 * W1[D] + b1
    w1b = np.ascontiguousarray(
        np.concatenate([W1[:D], bias1[None, :]], axis=0)).astype(BF)  # (33, 512)
    negw3t = np.ascontiguousarray(-W3.T).astype(BF)                   # (32, 512)
    w2p = [np.ascontiguousarray(np.concatenate(
               [W2[(2 * i) * 128:(2 * i + 1) * 128],
                W2[(2 * i + 1) * 128:(2 * i + 2) * 128]], axis=1)).astype(BF)
           for i in range(2)]                                          # (128, 1024)
    w3p = np.ascontiguousarray(
        W3.reshape(NK, 128, D).transpose(1, 0, 2).reshape(128, NK * D)
    ).astype(BF)

    Mt = (W3.astype(np.float64) @ W1[:D].astype(np.float64)).T   # M^T (H, H)
    vneg = (W2.astype(np.float64) * Mt).sum(axis=0)              # colsum of C
    v = np.zeros(16 * 66, dtype=np.float32)
    v[B2O:B2O + H] = b2
    v[VNO:VNO + H] = vneg.astype(np.float32)
    v[B3O:B3O + D] = b3
    brow = np.ascontiguousarray(v.astype(BF).reshape(16, 66))

    in_maps = []
    for i in range(NCORES):
        xs = x[i * BC:(i + 1) * BC, :D]
        xt = np.empty((D + 1, BC), dtype=BF)
        xt[0:D] = xs.T.astype(BF)
        xt[D] = BF(1.0)
        m = {"xt": np.ascontiguousarray(xt), "w1b": w1b, "negw3t": negw3t,
             "w3p": w3p, "brow": brow, "w2p0": w2p[0], "w2p1": w2p[1]}
        in_maps.append(m)
    return in_maps


def kernel(t, x, W1, b1, W2, b2, W3, b3):
    from concourse.bass_utils import run_bass_kernel_spmd

    nc = _get_nc()
    in_maps = _make_in_maps(t, x, W1, b1, W2, b2, W3, b3)
    res = run_bass_kernel_spmd(nc, in_maps, core_ids=list(range(NCORES)))
    parts = []
    for i in range(NCORES):
        dx = res.results[i]["out_dx"]        # (32, 256)
        dv = res.results[i]["out_div"]       # (1, 256)
        parts.append(np.concatenate([dx.T, dv.T], axis=1))
    return np.ascontiguousarray(np.concatenate(parts, axis=0))


# revision 26
# speedup vs baseline: 1.3939x; 1.1776x over previous
"""CNF vector-field + exact Jacobian-trace kernel for Trainium2 (8 NeuronCores).

Math: for each sample x (D=32), with inp = [x, t] (33,):
  h1 = tanh(inp @ W1 + b1); h2 = tanh(h1 @ W2 + b2); dx = h2 @ W3 + b3
  div = trace(J) = d1^T C d2,  C = W2 * (W3 @ W1r)^T,  d_i = 1 - h_i^2
  out = [dx, div]  (B, 33)

Implementation notes (hardware-measured constraints):
  - all matmul operands bf16 (tol 2e-2, measured ~4e-3): single-pass PE
    matmuls (fp32r is 2-pass) and half the DMA bytes
  - PSUM accumulation groups must NOT share a bank: a group's start=True
    clears the whole bank's has_written bits, so an interleaved second
    group makes the first overwrite instead of accumulate. One group per
    2KB bank; an 8-slot ring recycles banks (warm/z1 -> mp -> z2 -> gt ->
    dx/div) in dependency order.
  - DMA engines cost ~125ns per descriptor: plain 2-D row-per-descriptor
    transfers spread round-robin over all 16 engines, and 2KB descriptors
    reach ~250 GB/s/queue (1KB ~150). Rearranged/3-D patterns serialize
    onto ~2 engines — avoid. W2 row-chunks are DMA'd as column-paired
    (128, 1024) tiles so each descriptor is 2KB.
  - completion semaphores ride the same engines as data: keep every
    descriptor <=2KB and all transfers >=16 descriptors so no engine
    clogs and sems arrive with the data.
  - host pre-computes: x^T with a ones row (bias1 via K=33 matmul row),
    -W3^T, W3 row-chunk pack, vneg = colsum(W2 * M^T); output goes out
    untransposed and the host transposes back.
  - PE warm-up matmuls bridge the DMA wait: the HAM clock gate runs the PE
    at 1.2 GHz until ~3.4us of sustained activity, 2.4 GHz after.
"""
import sys

for _p in ("/opt/trn_rl_repo", "/root/.axon_site/_ro/trn_rl_repo"):
    if _p not in sys.path:
        sys.path.append(_p)

import numpy as np
import ml_dtypes

B, D, H = 2048, 32, 512
NCORES = 8
BC = B // NCORES          # 256 rows per core
NK = H // 128             # 4 chunks of the hidden dim
BF = ml_dtypes.bfloat16

# brow offsets (bf16, partition 0, 1056 = 16*66 elems)
B2O, VNO, B3O = 0, H, 2 * H

_CACHE = {}


def _build():
    import concourse.bass as bass  # noqa: F401
    import concourse.tile as tile
    from concourse import bacc, mybir

    f32 = mybir.dt.float32
    bf16 = mybir.dt.bfloat16
    AF = mybir.ActivationFunctionType
    ALU = mybir.AluOpType

    nc = bacc.Bacc("TRN2", target_bir_lowering=False, debug=False,
                   num_devices=NCORES)

    xw_ext = nc.dram_tensor("xw", [D + 1, BC + H], bf16,
                            kind="ExternalInput").ap()
    w2c_ext = [nc.dram_tensor(f"w2c{k}", [128, H], bf16,
                              kind="ExternalInput").ap() for k in range(2)]
    w2p1_ext = nc.dram_tensor("w2p1", [128, 2 * H], bf16,
                              kind="ExternalInput").ap()
    nw_ext = nc.dram_tensor("negw3t", [D, H], bf16,
                            kind="ExternalInput").ap()
    w3p_ext = nc.dram_tensor("w3p", [128, NK * D], bf16,
                             kind="ExternalInput").ap()
    brow_ext = nc.dram_tensor("brow", [16, 66], bf16,
                              kind="ExternalInput").ap()
    bcol_ext = nc.dram_tensor("bcol", [128, 9], f32,
                              kind="ExternalInput").ap()
    odx_ext = nc.dram_tensor("out_dx", [D, BC], f32,
                             kind="ExternalOutput").ap()
    odiv_ext = nc.dram_tensor("out_div", [1, BC], f32,
                              kind="ExternalOutput").ap()

    with tile.TileContext(nc) as tc:
        with tc.tile_pool(name="const", bufs=1) as cpool, \
             tc.tile_pool(name="work", bufs=1) as wpool, \
             tc.tile_pool(name="ps", bufs=1, space="PSUM") as pps:

            def ps_tile(nm, shape=(128, H)):
                return pps.tile(list(shape), f32, name=nm, tag="ring", bufs=8)

            # ---- on-device constants (no DMA) + ACT table preload ----
            wsrc = cpool.tile([128, H], bf16, name="wsrc")
            nc.vector.memset(wsrc[:, :], 0.0)
            ones_row = wpool.tile([1, BC], bf16, name="ones_row")
            nc.gpsimd.memset(ones_row[:, :], 1.0)
            neg_col = wpool.tile([128, 1], bf16, name="neg_col")
            nc.gpsimd.memset(neg_col[:, :], -1.0)
            dm0 = wpool.tile([1, 1], f32, name="dm0")
            nc.gpsimd.memset(dm0[:, :], 0.0)
            dm1 = wpool.tile([1, 1], f32, name="dm1")
            nc.scalar.activation(dm1[:, :], dm0[:, :], AF.Tanh)

            # ---- input DMAs: plain 2-D only, split across both queues ----
            # sync queue: z1 inputs, W2 pair 0 (rounds k0/k1), then a tiny
            # flush transfer — a DMA's last completion-sem increments post
            # while the NEXT transfer on its queue runs, so the flush pulls
            # w2p0's semaphore in right behind its data
            xw = cpool.tile([D + 1, BC + H], bf16, name="xw")
            nc.sync.dma_start(out=xw[:, :], in_=xw_ext[:, :])
            xt = xw[:, 0:BC]
            w1b = xw[:, BC:BC + H]
            w2c = [cpool.tile([128, H], bf16, name=f"w2c{k}")
                   for k in range(2)]
            nc.sync.dma_start(out=w2c[0][:, :], in_=w2c_ext[0][:, :])
            scrA = wpool.tile([1, 16 * 66], bf16, name="scrA")
            nc.sync.dma_start(
                out=scrA[:, :].rearrange("p (a b) -> p a b", a=16),
                in_=brow_ext.rearrange("(o a) b -> o a b", o=1))
            nc.sync.dma_start(out=w2c[1][:, :], in_=w2c_ext[1][:, :])
            scr = wpool.tile([1, 16 * 66], bf16, name="scr")
            nc.sync.dma_start(
                out=scr[:, :].rearrange("p (a b) -> p a b", a=16),
                in_=brow_ext.rearrange("(o a) b -> o a b", o=1))
            # scalar queue: W2 pair 1 (rounds k2/k3) and the late-needed rest
            w2p1 = cpool.tile([128, 2 * H], bf16, name="w2p1")
            nc.scalar.dma_start(out=w2p1[:, :], in_=w2p1_ext[:, :])
            negw3t = cpool.tile([D, H], bf16, name="negw3t")
            nc.scalar.dma_start(out=negw3t[:, :], in_=nw_ext[:, :])
            w3p = cpool.tile([128, NK * D], bf16, name="w3p")
            nc.scalar.dma_start(out=w3p[:, :], in_=w3p_ext[:, :])
            bcol = cpool.tile([128, 9], f32, name="bcol")
            nc.scalar.dma_start(out=bcol[:, :], in_=bcol_ext[:, :])
            w2k = [w2c[0][:, :], w2c[1][:, :],
                   w2p1[:, 0:H], w2p1[:, H:2 * H]]

            # ---- PE warm-up against the HAM clock gate ----
            # dense back-to-back N=256 matmuls from program entry until the
            # first input semaphores land (~2.2us)
            for i in range(10):
                wp = ps_tile(f"warm{i}", shape=(128, BC))
                nc.tensor.matmul(wp[:, :], wsrc[:, 0:128], wsrc[:, :],
                                 start=True, stop=True)

            # ---- PE pipeline warm-up: dense N=512 matmuls bridge the
            #      input-DMA wait (~9 x 430ns) ----
            warm_ps = [ps_tile(f"warm{i}") for i in range(8)]
            for i in range(6):
                nc.tensor.matmul(warm_ps[i % 8][:, :], wsrc[:, 0:128],
                                 wsrc[:, :], start=True, stop=True)

            # ---- z1 (K=33: bias1 folded in via the ones row of x^T) ----
            # ring slots 6,7; two single-MM groups per bank is safe (each
            # is start+stop in one instruction)
            z1t = [ps_tile(f"z1{i}") for i in range(2)]
            for m in range(NK):
                nc.tensor.matmul(
                    z1t[m // 2][:, (m % 2) * BC:(m % 2 + 1) * BC],
                    xw[0:D + 1, BC + m * 128:BC + (m + 1) * 128],
                    xw[0:D + 1, 0:BC], start=True, stop=True)
            h1t = [wpool.tile([128, 2 * BC], bf16, name=f"h1t{i}")
                   for i in range(2)]
            for i in range(2):
                nc.scalar.activation(h1t[i][:, :], z1t[i][:, :], AF.Tanh)

            # ---- mp = W1r^T @ (-W3^T) per row-chunk (slots 0-3);
            #      P = W2 * mp on DVE, interleaved with h1sq ----
            pmat = [cpool.tile([128, H], bf16, name=f"p{k}")
                    for k in range(NK)]
            h1sq = [wpool.tile([128, 2 * BC], bf16, name=f"h1sq{i}")
                    for i in range(2)]
            mps = [ps_tile(f"mp{k}") for k in range(NK)]
            for k in range(NK):
                nc.tensor.matmul(mps[k][:, :],
                                 xw[0:D, BC + k * 128:BC + (k + 1) * 128],
                                 negw3t[:, :], start=True, stop=True)
            nc.vector.tensor_tensor(out=h1sq[0][:, :], in0=h1t[0][:, :],
                                    in1=h1t[0][:, :], op=ALU.mult)
            nc.vector.tensor_tensor(out=pmat[0][:, :], in0=w2k[0][:, :],
                                    in1=mps[0][:, :], op=ALU.mult)
            nc.vector.tensor_tensor(out=h1sq[1][:, :], in0=h1t[1][:, :],
                                    in1=h1t[1][:, :], op=ALU.mult)
            for k in range(1, NK):
                nc.vector.tensor_tensor(out=pmat[k][:, :], in0=w2k[k][:, :],
                                        in1=mps[k][:, :], op=ALU.mult)

            # ---- z2 & gt accumulation rounds (k-outer) ----
            # one group per bank: z2 slots 4,5,6,7 / gt slots 0,1,2,3
            z2t = [ps_tile(f"z2{m}", shape=(128, BC)) for m in range(NK)]
            gtt = [ps_tile(f"gt{m}", shape=(128, BC)) for m in range(NK)]
            for k in range(NK):
                hk = h1t[k // 2][:, (k % 2) * BC:(k % 2 + 1) * BC]
                sk = h1sq[k // 2][:, (k % 2) * BC:(k % 2 + 1) * BC]
                for m in range(NK):
                    nc.tensor.matmul(z2t[m][:, :],
                                     w2k[k][:, m * 128:(m + 1) * 128], hk,
                                     start=(k == 0), stop=False)
                if k == NK - 1:
                    # close z2 groups first so tanh2 overlaps gt round 3
                    for m in range(NK):
                        nc.tensor.matmul(z2t[m][:, :],
                                         brow[:, B2O + m * 128:B2O + (m + 1) * 128],
                                         ones_row, start=False, stop=True)
                for m in range(NK):
                    nc.tensor.matmul(gtt[m][:, :],
                                     pmat[k][:, m * 128:(m + 1) * 128], sk,
                                     start=(k == 0), stop=False)
                if k == NK - 1:
                    for m in range(NK):
                        nc.tensor.matmul(gtt[m][:, :],
                                         brow[:, VNO + m * 128:VNO + (m + 1) * 128],
                                         ones_row, start=False, stop=True)

            # ---- tanh2 per chunk, h2sq (GpSimd+DVE), E ----
            h2t = [wpool.tile([128, 2 * BC], bf16, name=f"h2t{i}")
                   for i in range(2)]
            for m in range(NK):
                nc.scalar.activation(h2t[m // 2][:, (m % 2) * BC:(m % 2 + 1) * BC],
                                     z2t[m][:, :], AF.Tanh,
                                     bias=bcol[:, m:m + 1])
            h2sq = [wpool.tile([128, 2 * BC], bf16, name=f"h2sq{i}")
                    for i in range(2)]
            for m in range(2):
                nc.gpsimd.tensor_tensor(
                    out=h2sq[m // 2][:, (m % 2) * BC:(m % 2 + 1) * BC],
                    in0=h2t[m // 2][:, (m % 2) * BC:(m % 2 + 1) * BC],
                    in1=h2t[m // 2][:, (m % 2) * BC:(m % 2 + 1) * BC],
                    op=ALU.mult)
            for m in range(2, NK):
                nc.vector.tensor_tensor(
                    out=h2sq[m // 2][:, (m % 2) * BC:(m % 2 + 1) * BC],
                    in0=h2t[m // 2][:, (m % 2) * BC:(m % 2 + 1) * BC],
                    in1=h2t[m // 2][:, (m % 2) * BC:(m % 2 + 1) * BC],
                    op=ALU.mult)
            ee = [wpool.tile([128, 2 * BC], bf16, name=f"ee{i}")
                  for i in range(2)]
            for m in range(NK):
                nc.vector.scalar_tensor_tensor(
                    out=ee[m // 2][:, (m % 2) * BC:(m % 2 + 1) * BC],
                    in0=h2sq[m // 2][:, (m % 2) * BC:(m % 2 + 1) * BC],
                    scalar=1.0, in1=gtt[m][:, :],
                    op0=ALU.subtract, op1=ALU.mult)

            # ---- div = (-1)^T E first (it feeds the last output DMA),
            #      then dx = W3^T h2 + b3 ----
            div_ps = ps_tile("div", shape=(1, BC))
            for k in range(NK):
                nc.tensor.matmul(div_ps[:, :], neg_col[:, :],
                                 ee[k // 2][:, (k % 2) * BC:(k % 2 + 1) * BC],
                                 start=(k == 0), stop=(k == NK - 1))
            dx_ps = ps_tile("dx", shape=(D, BC))
            for k in range(NK):
                nc.tensor.matmul(dx_ps[:, :], w3p[:, k * D:(k + 1) * D],
                                 h2t[k // 2][:, (k % 2) * BC:(k % 2 + 1) * BC],
                                 start=(k == 0), stop=(k == NK - 1))

            # ---- stage on DVE (ACT is busy with tanh2), store on both
            #      queues in parallel; div first (it completes first) ----
            odiv = wpool.tile([1, BC], f32, name="odiv")
            nc.vector.tensor_copy(odiv[:, :], div_ps[:, :])
            nc.scalar.dma_start(out=odiv_ext[:, :], in_=odiv[:, :])
            odx = wpool.tile([D, BC], f32, name="odx")
            nc.vector.tensor_scalar(out=odx[:, :], in0=dx_ps[:, :],
                                    scalar1=bcol[0:D, 8:9], scalar2=None,
                                    op0=ALU.add)
            nc.sync.dma_start(out=odx_ext[:, :], in_=odx[:, :])

    nc.compile()
    return nc


def _get_nc():
    if "nc" not in _CACHE:
        _CACHE["nc"] = _build()
    return _CACHE["nc"]


def _make_in_maps(t, x, W1, b1, W2, b2, W3, b3):
    t0 = np.float32(np.asarray(t, np.float32).ravel()[0])
    x = np.asarray(x, np.float32)
    W1 = np.asarray(W1, np.float32)
    b1 = np.asarray(b1, np.float32)
    W2 = np.asarray(W2, np.float32)
    b2 = np.asarray(b2, np.float32)
    W3 = np.asarray(W3, np.float32)
    b3 = np.asarray(b3, np.float32)

    bias1 = t0 * W1[D] + b1
    w1b = np.ascontiguousarray(
        np.concatenate([W1[:D], bias1[None, :]], axis=0)).astype(BF)  # (33, 512)
    negw3t = np.ascontiguousarray(-W3.T).astype(BF)                   # (32, 512)
    w2c = [np.ascontiguousarray(W2[k * 128:(k + 1) * 128]).astype(BF)
           for k in range(2)]                                          # (128, 512)
    w2p1 = np.ascontiguousarray(np.concatenate(
        [W2[2 * 128:3 * 128], W2[3 * 128:4 * 128]], axis=1)).astype(BF)
    w3p = np.ascontiguousarray(
        W3.reshape(NK, 128, D).transpose(1, 0, 2).reshape(128, NK * D)
    ).astype(BF)

    Mt = (W3.astype(np.float64) @ W1[:D].astype(np.float64)).T   # M^T (H, H)
    vneg = (W2.astype(np.float64) * Mt).sum(axis=0)              # colsum of C
    v = np.zeros(16 * 66, dtype=np.float32)
    v[VNO:VNO + H] = vneg.astype(np.float32)
    brow = np.ascontiguousarray(v.astype(BF).reshape(16, 66))   # flush + vneg
    bcol = np.zeros((128, 9), dtype=np.float32)
    bcol[:, 0:4] = b2.reshape(NK, 128).T
    bcol[0:D, 8] = b3

    in_maps = []
    for i in range(NCORES):
        xs = x[i * BC:(i + 1) * BC, :D]
        xw = np.empty((D + 1, BC + H), dtype=BF)
        xw[0:D, 0:BC] = xs.T.astype(BF)
        xw[D, 0:BC] = BF(1.0)
        xw[:, BC:BC + H] = w1b
        m = {"xw": np.ascontiguousarray(xw), "negw3t": negw3t,
             "w3p": w3p, "brow": brow, "bcol": bcol,
             "w2c0": w2c[0], "w2c1": w2c[1], "w2p1": w2p1}
        in_maps.append(m)
    return in_maps


def kernel(t, x, W1, b1, W2, b2, W3, b3):
    from concourse.bass_utils import run_bass_kernel_spmd

    nc = _get_nc()
    in_maps = _make_in_maps(t, x, W1, b1, W2, b2, W3, b3)
    res = run_bass_kernel_spmd(nc, in_maps, core_ids=list(range(NCORES)))
    parts = []
    for i in range(NCORES):
        dx = res.results[i]["out_dx"]        # (32, 256)
        dv = res.results[i]["out_div"]       # (1, 256)
        parts.append(np.concatenate([dx.T, dv.T], axis=1))
    return np.ascontiguousarray(np.concatenate(parts, axis=0))
